# revision 19
# baseline (speedup 1.0000x reference)
"""Trainium2 Bass kernel for nn_AttentionAggregator3d.

Math (per batch b):
    zmf = zm.reshape(CM, N)                     # N = D*W*H = 4096 tokens
    q = Wq @ zmf + bq ; k = Wk @ zmf + bk       # (16, N)
    v = Wv @ zmf + bv                           # (128, N)
    A = softmax_n(q^T k)                        # (N, N), softmax over keys n
    out = v @ A^T ; result = zc + gamma * out

Key transformations used by the kernel:
  * logits = zmf^T G zmf (+ key-side bias term) with G = Wq^T Wk precomputed
    on host, turning the K=16 contraction into a full K=128 PE contraction.
  * bq/bk only affect softmax through the per-key term r[n] = (Wk^T bq)·zm[:,n]
    (per-query terms cancel in softmax); handled as a per-partition exp bias.
  * Sharding: 8 cores = batch (2) x query-block (4, 1024 queries each). Each
    core sees its batch's zm rotated so its query block sits at columns 0:1024
    (softmax/PV sum over all keys, so key order is irrelevant).
  * Layout: exp'd scores E^T are kept (keys on partitions, queries free) so
    the PV matmul contracts over keys on the PE, all in float32r (full-rate
    fp32-class matmuls). Softmax denominators are accumulated partly by PE
    ones-matmuls into a single PSUM bank (two m-halves parked on partitions
    0 and 32), partly by DVE adds (tunable split), folded, inverted via
    exp(-ln s) (kept inside one ACT table set), broadcast with a K=1 matmul,
    and applied with gamma / zc in a halved, pipelined DVE tail.
"""

import os
import sys
import types

import ml_dtypes
import numpy as np

import concourse.bacc as bacc_mod
import concourse.tile as tile
from concourse import mybir
from concourse.bass_utils import run_bass_kernel_spmd

B, CC, CM, P = 2, 128, 128, 16
N = 16 * 16 * 16          # 4096 tokens
MBLK = N // 4             # 1024 queries per core
NCORES = 8
NCHUNK = N // 128         # 32 key chunks of 128

F32 = mybir.dt.float32
F32R = mybir.dt.float32r
BF16 = mybir.dt.bfloat16
AF = mybir.ActivationFunctionType
ALU = mybir.AluOpType

# chunks whose softmax-denominator contribution is summed on the PE
# (ones-matmul); the rest accumulate on the DVE. Tunable.
PE_SUM_PERIOD = int(os.environ.get("BASS_PE_SUM_PERIOD", "2"))

LAST_RESULTS = None  # BassKernelResults of the most recent run (for test.py)


def _ensure_ntff_hook() -> bool:
    """The grading image lacks antenv.axon_hooks; synthesize it from the
    boot module's ctypes NTFF driver so trace=True works under axon."""
    try:
        import antenv.axon_hooks  # noqa: F401

        return True
    except ImportError:
        pass
    try:
        import antenv
        from trn_agent_boot.trn_boot import _ntff_profile_via_ctypes

        hook = _ntff_profile_via_ctypes("/opt/axon/libaxon_pjrt.so")
        mod = types.ModuleType("antenv.axon_hooks")
        mod.get_axon_ntff_profile_hook = lambda: hook
        mod.set_axon_ntff_profile_hook = lambda h: None
        sys.modules["antenv.axon_hooks"] = mod
        antenv.axon_hooks = mod
        return hook is not None
    except Exception:
        return False


# Route Exp and Ln to the one table set that holds both, so the kernel pays a
# single ACT_TABLE_LOAD instead of three (exp -> ln -> exp again).
_orig_gat = bacc_mod.get_activation_tables
_COMBINED_SET = "natural_log_exp_and_others"


def _patched_gat(arch):
    tabs = _orig_gat(arch)
    if _COMBINED_SET in tabs:
        for name, fns in tabs.items():
            if name != _COMBINED_SET:
                fns.discard(AF.Exp)
                fns.discard(AF.Ln)
    return tabs


bacc_mod.get_activation_tables = _patched_gat


if os.environ.get("BASS_LDW_OPT", "0") == "1":
    import subprocess as _sp

    _orig_run = _sp.run

    def _patched_run(cmd, *a, **kw):
        if isinstance(cmd, list) and any(
            isinstance(c, str) and "enable-ldw-opt" in c for c in cmd
        ):
            cmd = [
                c.replace("--enable-ldw-opt=false", "--enable-ldw-opt=true")
                if isinstance(c, str)
                else c
                for c in cmd
            ]
        return _orig_run(cmd, *a, **kw)

    _sp.run = _patched_run


def _build(use_qk_bias: bool):
    nc = bacc_mod.Bacc(
        "TRN2",
        target_bir_lowering=False,
        debug=False,
        num_devices=NCORES,
    )

    zm_d = nc.dram_tensor("zm", (CM, N), F32R, kind="ExternalInput").ap()
    zc_d = nc.dram_tensor("zc", (CC, MBLK), F32, kind="ExternalInput").ap()
    gt_d = nc.dram_tensor("gt", (CM, CM), F32R, kind="ExternalInput").ap()
    wvt_d = nc.dram_tensor("wvt", (CM, CC), BF16, kind="ExternalInput").ap()
    gam_d = nc.dram_tensor("gam", (CC, 1), F32, kind="ExternalInput").ap()
    adv_d = nc.dram_tensor("adv", (CC, 1), F32, kind="ExternalInput").ap()
    onesc_d = nc.dram_tensor("onesc", (128, 1), F32R, kind="ExternalInput").ap()
    onesr_d = nc.dram_tensor("onesr", (1, 128), F32R, kind="ExternalInput").ap()
    if use_qk_bias:
        u_d = nc.dram_tensor("u", (CM, 1), F32R, kind="ExternalInput").ap()
    out_d = nc.dram_tensor("out", (CC, MBLK), F32, kind="ExternalOutput").ap()

    pe_sum = [j for j in range(NCHUNK) if j % PE_SUM_PERIOD == PE_SUM_PERIOD - 1]
    dve_sum = [j for j in range(NCHUNK) if j not in pe_sum]

    with tile.TileContext(nc) as tc:
        with (
            tc.tile_pool(name="consts", bufs=1) as consts,
            tc.tile_pool(name="epool", bufs=6) as epool,
            tc.tile_pool(name="lpool", bufs=2, space="PSUM") as lpool,
            tc.tile_pool(name="opool", bufs=1, space="PSUM") as opool,
            tc.tile_pool(name="spool", bufs=1, space="PSUM") as spool,
        ):
            zm_sb = consts.tile([CM, N], F32R, tag="zm")
            zm_bf = consts.tile([CM, N], BF16, tag="zmbf")
            t_sb = consts.tile([CM, N], F32R, tag="t")
            vt_sb = consts.tile([128, N], F32R, tag="vt")  # chunk j at cols 128j
            zc_sb = consts.tile([CC, MBLK], F32, tag="zc")
            gt_sb = consts.tile([CM, CM], F32R, tag="gt")
            wvt_sb = consts.tile([CM, CC], BF16, tag="wvt")
            gam_sb = consts.tile([CC, 1], F32, tag="gam")
            adv_sb = consts.tile([CC, 1], F32, tag="adv")
            ones_col = consts.tile([128, 1], F32R, tag="onesc")
            ones_row = consts.tile([1, 128], F32R, tag="onesr")
            acc = consts.tile([128, MBLK], F32R, tag="acc")
            lns = consts.tile([1, MBLK], F32, tag="lns")
            rvec = consts.tile([1, MBLK], F32R, tag="rvec")
            rb_sb = consts.tile([128, MBLK], F32, tag="rb")
            tmp_sb = consts.tile([CC, MBLK], F32, tag="tmp")
            out_sb = consts.tile([CC, MBLK], F32, tag="outsb")
            if use_qk_bias:
                u_sb = consts.tile([CM, 1], F32R, tag="u")
                rn_sb = consts.tile([128, NCHUNK], F32, tag="rn")

            # ---- input DMAs, fanned across idle engine sequencers: each
            # dma_start costs ~0.6us of issue time on its sequencer, so
            # serializing them on one engine would delay the first matmul ----
            nc.scalar.dma_start(gt_sb[:], gt_d)
            nc.sync.dma_start(zm_sb[:, 0:512], zm_d[:, 0:512])
            nc.sync.dma_start(zm_sb[:, 512:1024], zm_d[:, 512:1024])
            nc.scalar.dma_start(ones_col[:], onesc_d)
            nc.gpsimd.dma_start(zm_sb[:, 1024:2048], zm_d[:, 1024:2048])
            nc.gpsimd.dma_start(zm_sb[:, 2048:3072], zm_d[:, 2048:3072])
            nc.gpsimd.dma_start(zm_sb[:, 3072:4096], zm_d[:, 3072:4096])
            nc.scalar.dma_start(wvt_sb[:], wvt_d)
            nc.sync.dma_start(ones_row[:], onesr_d)
            nc.sync.dma_start(gam_sb[:], gam_d)
            nc.sync.dma_start(adv_sb[:], adv_d)
            if use_qk_bias:
                nc.gpsimd.dma_start(u_sb[:], u_d)
            nc.sync.dma_start(zc_sb[:], zc_d)

            out_ps = opool.tile([CC, MBLK], F32, tag="out")
            # one PSUM bank: m-half h sums parked on partition 32h
            s_ps = spool.tile([1, MBLK], F32, tag="s")

            def emit_t_piece(i):
                # t[:, 512i:512(i+1)] = G @ zm[:, ...] (covers chunks 4i..4i+3)
                tps = lpool.tile([128, 512], F32, tag="L")
                nc.tensor.matmul(
                    tps[:],
                    gt_sb[:],
                    zm_sb[:, i * 512 : (i + 1) * 512],
                    start=True,
                    stop=True,
                )
                nc.vector.tensor_copy(t_sb[:, i * 512 : (i + 1) * 512], tps[:])

            def emit_vt_batch(i):
                # vt chunk j = (zm chunk j)^T @ Wv^T for j in 4i..4i+3
                nc.vector.tensor_copy(
                    zm_bf[:, i * 512 : (i + 1) * 512],
                    zm_sb[:, i * 512 : (i + 1) * 512].bitcast(F32),
                )
                vps = lpool.tile([128, 512], F32, tag="L")
                for k in range(4):
                    j = 4 * i + k
                    nc.tensor.matmul(
                        vps[:, 128 * k : 128 * (k + 1)],
                        zm_bf[:, 128 * j : 128 * (j + 1)],
                        wvt_sb[:],
                        start=True,
                        stop=True,
                    )
                nc.vector.tensor_copy(vt_sb[:, i * 512 : (i + 1) * 512], vps[:])
                if use_qk_bias:
                    rnps = lpool.tile([128, 4], F32, tag="L")
                    for k in range(4):
                        j = 4 * i + k
                        nc.tensor.matmul(
                            rnps[:, k : k + 1],
                            zm_sb[:, 128 * j : 128 * (j + 1)],
                            u_sb[:],
                            start=True,
                            stop=True,
                        )
                    nc.vector.tensor_copy(rn_sb[:, 4 * i : 4 * (i + 1)], rnps[:])

            emit_t_piece(0)

            e_tiles = {}
            first_pe = pe_sum[0] if pe_sum else None
            first_dve = dve_sum[0] if dve_sum else None

            for j in range(NCHUNK + 1):
                if j < NCHUNK:
                    if j % 4 == 1 and j // 4 + 1 <= 7:
                        emit_t_piece(j // 4 + 1)
                    if j % 4 == 2 and j // 4 + 1 <= 7:
                        emit_vt_batch(j // 4 + 1)
                    # logits^T chunk j: (keys 128, queries 1024)
                    lps = lpool.tile([128, MBLK], F32, tag="L")
                    for h in range(2):
                        nc.tensor.matmul(
                            lps[:, h * 512 : (h + 1) * 512],
                            t_sb[:, 128 * j : 128 * (j + 1)],
                            zm_sb[:, h * 512 : (h + 1) * 512],
                            start=True,
                            stop=True,
                        )
                    ej = epool.tile([128, MBLK], F32R, tag="E")
                    bias = rn_sb[:, j : j + 1] if use_qk_bias else 0.0
                    nc.scalar.activation(ej[:], lps[:], AF.Exp, bias=bias)
                    e_tiles[j] = ej
                    if j == 0:
                        emit_vt_batch(0)
                if j >= 1:
                    jj = j - 1
                    ej = e_tiles.pop(jj)
                    for h in range(2):
                        nc.tensor.matmul(
                            out_ps[:, h * 512 : (h + 1) * 512],
                            vt_sb[:, 128 * jj : 128 * (jj + 1)],
                            ej[:, h * 512 : (h + 1) * 512],
                            start=(jj == 0),
                            stop=(jj == NCHUNK - 1),
                        )
                    if jj in pe_sum:
                        for h in range(2):
                            nc.tensor.matmul(
                                s_ps[0:1, h * 512 : (h + 1) * 512],
                                ones_col[:],
                                ej[:, h * 512 : (h + 1) * 512],
                                start=(jj == first_pe),
                                stop=False,
                                skip_group_check=True,
                            )
                    else:
                        if jj == first_dve:
                            nc.vector.tensor_copy(acc[:], ej[:])
                        else:
                            nc.vector.tensor_add(acc[:], acc[:], ej[:])

            # tail, in halves so ln/exp/broadcast/final/DMA pipeline
            for h in range(2):
                sl = slice(h * 512, (h + 1) * 512)
                ph = slice(0, 1)
                # fold the DVE accumulator into s (cross-partition reduce)
                nc.tensor.matmul(
                    s_ps[0:1, sl],
                    ones_col[:],
                    acc[:, sl],
                    start=(first_pe is None),
                    stop=True,
                    skip_group_check=True,
                )
                # r = 1/s via exp(-ln s): same ACT table set as the main exps
                nc.scalar.activation(lns[:, sl], s_ps[:, sl], AF.Ln)
                nc.scalar.activation(rvec[:, sl], lns[:, sl], AF.Exp, scale=-1.0)
                # broadcast r across partitions with a K=1 matmul, fold gamma
                rb_ps = lpool.tile([128, 512], F32, tag="L")
                nc.tensor.matmul(
                    rb_ps[:], ones_row[:], rvec[:, sl], start=True, stop=True
                )
                nc.vector.tensor_scalar(
                    out=rb_sb[:, sl],
                    in0=rb_ps[:],
                    scalar1=gam_sb[:, 0:1],
                    scalar2=None,
                    op0=ALU.mult,
                )
                # out = zc + (outPV * gamma/s + gamma*bv)
                nc.vector.tensor_tensor(
                    tmp_sb[:, sl], out_ps[:, sl], rb_sb[:, sl], op=ALU.mult
                )
                nc.vector.scalar_tensor_tensor(
                    out_sb[:, sl],
                    tmp_sb[:, sl],
                    adv_sb[:, 0:1],
                    zc_sb[:, sl],
                    op0=ALU.add,
                    op1=ALU.add,
                )
                nc.sync.dma_start(out_d[:, sl], out_sb[:, sl])

    nc.compile()
    return nc


_CACHE = {}


def _get_program(use_qk_bias: bool):
    if use_qk_bias not in _CACHE:
        _CACHE[use_qk_bias] = _build(use_qk_bias)
    return _CACHE[use_qk_bias]


def kernel(zc, zm, Wq, bq, Wk, bk, Wv, bv, gamma):
    global LAST_RESULTS
    zc = np.ascontiguousarray(zc, dtype=np.float32)
    zm = np.ascontiguousarray(zm, dtype=np.float32)
    zmf = zm.reshape(B, CM, N)
    zcf = zc.reshape(B, CC, N)

    Wq = np.asarray(Wq, dtype=np.float32)
    Wk = np.asarray(Wk, dtype=np.float32)
    Wv = np.asarray(Wv, dtype=np.float32)
    gt = (Wk.astype(np.float64).T @ Wq.astype(np.float64)).astype(np.float32)
    wvt = np.ascontiguousarray(Wv.T).astype(ml_dtypes.bfloat16)
    gamma_v = np.float32(np.asarray(gamma).reshape(-1)[0])
    gam_arr = np.full((CC, 1), gamma_v, dtype=np.float32)
    adv_arr = (gamma_v * np.asarray(bv, dtype=np.float32)).reshape(CC, 1)
    adv_arr = np.ascontiguousarray(adv_arr)

    use_qk_bias = bool(np.any(bq)) or bool(np.any(bk))
    nc = _get_program(use_qk_bias)

    in_maps = []
    for c in range(NCORES):
        b, jblk = divmod(c, 4)
        m = {
            "zm": np.ascontiguousarray(np.roll(zmf[b], -MBLK * jblk, axis=1)),
            "zc": np.ascontiguousarray(zcf[b][:, MBLK * jblk : MBLK * (jblk + 1)]),
            "gt": gt,
            "wvt": wvt,
            "gam": gam_arr,
            "adv": adv_arr,
            "onesc": np.ones((128, 1), dtype=np.float32),
            "onesr": np.ones((1, 128), dtype=np.float32),
        }
        if use_qk_bias:
            m["u"] = np.ascontiguousarray(
                (Wk.T @ np.asarray(bq, dtype=np.float32)).reshape(CM, 1)
            )
        in_maps.append(m)

    trace = bool(int(os.environ.get("BASS_KERNEL_TRACE", "0")))
    if trace and not _ensure_ntff_hook():
        trace = False
    res = run_bass_kernel_spmd(
        nc,
        in_maps,
        core_ids=list(range(NCORES)),
        trace=trace,
    )
    LAST_RESULTS = res

    out = np.empty((B, CC, N), dtype=np.float32)
    for c in range(NCORES):
        b, jblk = divmod(c, 4)
        out[b][:, MBLK * jblk : MBLK * (jblk + 1)] = res.results[c]["out"]
    return out.reshape(zc.shape)


# revision 21
# speedup vs baseline: 1.0048x; 1.0048x over previous
"""Trainium2 Bass kernel for nn_AttentionAggregator3d.

Math (per batch b):
    zmf = zm.reshape(CM, N)                     # N = D*W*H = 4096 tokens
    q = Wq @ zmf + bq ; k = Wk @ zmf + bk       # (16, N)
    v = Wv @ zmf + bv                           # (128, N)
    A = softmax_n(q^T k)                        # (N, N), softmax over keys n
    out = v @ A^T ; result = zc + gamma * out

Key transformations used by the kernel:
  * logits = zmf^T G zmf (+ key-side bias term) with G = Wq^T Wk precomputed
    on host, turning the K=16 contraction into a full K=128 PE contraction.
  * bq/bk only affect softmax through the per-key term r[n] = (Wk^T bq)·zm[:,n]
    (per-query terms cancel in softmax); handled as a per-partition exp bias.
  * Sharding: 8 cores = batch (2) x query-block (4, 1024 queries each). Each
    core sees its batch's zm rotated so its query block sits at columns 0:1024
    (softmax/PV sum over all keys, so key order is irrelevant).
  * Layout: exp'd scores E^T are kept (keys on partitions, queries free) so
    the PV matmul contracts over keys on the PE, all in float32r (full-rate
    fp32-class matmuls). Softmax denominators are accumulated partly by PE
    ones-matmuls into a single PSUM bank (two m-halves parked on partitions
    0 and 32), partly by DVE adds (tunable split), folded, inverted via
    exp(-ln s) (kept inside one ACT table set), broadcast with a K=1 matmul,
    and applied with gamma / zc in a halved, pipelined DVE tail.
"""

import os
import sys
import types

import ml_dtypes
import numpy as np

import concourse.bacc as bacc_mod
import concourse.tile as tile
from concourse import mybir
from concourse.bass_utils import run_bass_kernel_spmd

B, CC, CM, P = 2, 128, 128, 16
N = 16 * 16 * 16          # 4096 tokens
MBLK = N // 4             # 1024 queries per core
NCORES = 8
NCHUNK = N // 128         # 32 key chunks of 128

F32 = mybir.dt.float32
F32R = mybir.dt.float32r
BF16 = mybir.dt.bfloat16
AF = mybir.ActivationFunctionType
ALU = mybir.AluOpType

# chunks whose softmax-denominator contribution is summed on the PE
# (ones-matmul); the rest accumulate on the DVE. Tunable.
PE_SUM_PERIOD = int(os.environ.get("BASS_PE_SUM_PERIOD", "2"))

LAST_RESULTS = None  # BassKernelResults of the most recent run (for test.py)


def _ensure_ntff_hook() -> bool:
    """The grading image lacks antenv.axon_hooks; synthesize it from the
    boot module's ctypes NTFF driver so trace=True works under axon."""
    try:
        import antenv.axon_hooks  # noqa: F401

        return True
    except ImportError:
        pass
    try:
        import antenv
        from trn_agent_boot.trn_boot import _ntff_profile_via_ctypes

        hook = _ntff_profile_via_ctypes("/opt/axon/libaxon_pjrt.so")
        mod = types.ModuleType("antenv.axon_hooks")
        mod.get_axon_ntff_profile_hook = lambda: hook
        mod.set_axon_ntff_profile_hook = lambda h: None
        sys.modules["antenv.axon_hooks"] = mod
        antenv.axon_hooks = mod
        return hook is not None
    except Exception:
        return False


# Route Exp and Ln to the one table set that holds both, so the kernel pays a
# single ACT_TABLE_LOAD instead of three (exp -> ln -> exp again).
_orig_gat = bacc_mod.get_activation_tables
_COMBINED_SET = "natural_log_exp_and_others"


def _patched_gat(arch):
    tabs = _orig_gat(arch)
    if _COMBINED_SET in tabs:
        for name, fns in tabs.items():
            if name != _COMBINED_SET:
                fns.discard(AF.Exp)
                fns.discard(AF.Ln)
    return tabs


bacc_mod.get_activation_tables = _patched_gat


if os.environ.get("BASS_LDW_OPT", "0") == "1":
    import subprocess as _sp

    _orig_run = _sp.run

    def _patched_run(cmd, *a, **kw):
        if isinstance(cmd, list) and any(
            isinstance(c, str) and "enable-ldw-opt" in c for c in cmd
        ):
            cmd = [
                c.replace("--enable-ldw-opt=false", "--enable-ldw-opt=true")
                if isinstance(c, str)
                else c
                for c in cmd
            ]
        return _orig_run(cmd, *a, **kw)

    _sp.run = _patched_run


def _build(use_qk_bias: bool):
    nc = bacc_mod.Bacc(
        "TRN2",
        target_bir_lowering=False,
        debug=False,
        num_devices=NCORES,
    )

    zm_d = nc.dram_tensor("zm", (CM, N), F32R, kind="ExternalInput").ap()
    zc_d = nc.dram_tensor("zc", (CC, MBLK), F32, kind="ExternalInput").ap()
    gt_d = nc.dram_tensor("gt", (CM, CM), F32R, kind="ExternalInput").ap()
    wvt_d = nc.dram_tensor("wvt", (CM, CC), BF16, kind="ExternalInput").ap()
    gam_d = nc.dram_tensor("gam", (CC, 1), F32, kind="ExternalInput").ap()
    adv_d = nc.dram_tensor("adv", (CC, 1), F32, kind="ExternalInput").ap()
    onesc_d = nc.dram_tensor("onesc", (128, 1), F32R, kind="ExternalInput").ap()
    onesr_d = nc.dram_tensor("onesr", (1, 128), F32R, kind="ExternalInput").ap()
    if use_qk_bias:
        u_d = nc.dram_tensor("u", (CM, 1), F32R, kind="ExternalInput").ap()
    out_d = nc.dram_tensor("out", (CC, MBLK), F32, kind="ExternalOutput").ap()

    pe_sum = [j for j in range(NCHUNK) if j % PE_SUM_PERIOD == PE_SUM_PERIOD - 1]
    dve_sum = [j for j in range(NCHUNK) if j not in pe_sum]

    with tile.TileContext(nc) as tc:
        with (
            tc.tile_pool(name="consts", bufs=1) as consts,
            tc.tile_pool(name="epool", bufs=6) as epool,
            tc.tile_pool(name="lpool", bufs=2, space="PSUM") as lpool,
            tc.tile_pool(name="opool", bufs=1, space="PSUM") as opool,
            tc.tile_pool(name="spool", bufs=1, space="PSUM") as spool,
        ):
            zm_sb = consts.tile([CM, N], F32R, tag="zm")
            zm_bf = consts.tile([CM, N], BF16, tag="zmbf")
            t_sb = consts.tile([CM, N], F32R, tag="t")
            vt_sb = consts.tile([128, N], F32R, tag="vt")  # chunk j at cols 128j
            zc_sb = consts.tile([CC, MBLK], F32, tag="zc")
            gt_sb = consts.tile([CM, CM], F32R, tag="gt")
            wvt_sb = consts.tile([CM, CC], BF16, tag="wvt")
            gam_sb = consts.tile([CC, 1], F32, tag="gam")
            adv_sb = consts.tile([CC, 1], F32, tag="adv")
            ones_col = consts.tile([128, 1], F32R, tag="onesc")
            ones_row = consts.tile([1, 128], F32R, tag="onesr")
            acc = consts.tile([128, MBLK], F32R, tag="acc")
            lns = consts.tile([1, MBLK], F32, tag="lns")
            rvec = consts.tile([1, MBLK], F32R, tag="rvec")
            rb_sb = consts.tile([128, MBLK], F32, tag="rb")
            tmp_sb = consts.tile([CC, MBLK], F32, tag="tmp")
            out_sb = consts.tile([CC, MBLK], F32, tag="outsb")
            if use_qk_bias:
                u_sb = consts.tile([CM, 1], F32R, tag="u")
                rn_sb = consts.tile([128, NCHUNK], F32, tag="rn")

            # ---- input DMAs, fanned across idle engine sequencers: each
            # dma_start costs ~0.6us of issue time on its sequencer, so
            # serializing them on one engine would delay the first matmul ----
            nc.scalar.dma_start(gt_sb[:], gt_d)
            nc.sync.dma_start(zm_sb[:, 0:512], zm_d[:, 0:512])
            nc.sync.dma_start(zm_sb[:, 512:1024], zm_d[:, 512:1024])
            nc.scalar.dma_start(ones_col[:], onesc_d)
            nc.gpsimd.dma_start(zm_sb[:, 1024:2048], zm_d[:, 1024:2048])
            nc.gpsimd.dma_start(zm_sb[:, 2048:3072], zm_d[:, 2048:3072])
            nc.gpsimd.dma_start(zm_sb[:, 3072:4096], zm_d[:, 3072:4096])
            nc.scalar.dma_start(wvt_sb[:], wvt_d)
            nc.sync.dma_start(ones_row[:], onesr_d)
            nc.sync.dma_start(gam_sb[:], gam_d)
            nc.sync.dma_start(adv_sb[:], adv_d)
            if use_qk_bias:
                nc.gpsimd.dma_start(u_sb[:], u_d)
            nc.sync.dma_start(zc_sb[:], zc_d)

            out_ps = opool.tile([CC, MBLK], F32, tag="out")
            # one PSUM bank: m-half h sums parked on partition 32h
            s_ps = spool.tile([1, MBLK], F32, tag="s")

            def emit_t_piece(i):
                # t[:, 512i:512(i+1)] = G @ zm[:, ...] (covers chunks 4i..4i+3)
                tps = lpool.tile([128, 512], F32, tag="L")
                nc.tensor.matmul(
                    tps[:],
                    gt_sb[:],
                    zm_sb[:, i * 512 : (i + 1) * 512],
                    start=True,
                    stop=True,
                )
                nc.scalar.copy(t_sb[:, i * 512 : (i + 1) * 512], tps[:])

            def emit_vt_batch(i):
                # vt chunk j = (zm chunk j)^T @ Wv^T for j in 4i..4i+3
                nc.vector.tensor_copy(
                    zm_bf[:, i * 512 : (i + 1) * 512],
                    zm_sb[:, i * 512 : (i + 1) * 512].bitcast(F32),
                )
                vps = lpool.tile([128, 512], F32, tag="L")
                for k in range(4):
                    j = 4 * i + k
                    nc.tensor.matmul(
                        vps[:, 128 * k : 128 * (k + 1)],
                        zm_bf[:, 128 * j : 128 * (j + 1)],
                        wvt_sb[:],
                        start=True,
                        stop=True,
                    )
                nc.vector.tensor_copy(vt_sb[:, i * 512 : (i + 1) * 512], vps[:])
                if use_qk_bias:
                    rnps = lpool.tile([128, 4], F32, tag="L")
                    for k in range(4):
                        j = 4 * i + k
                        nc.tensor.matmul(
                            rnps[:, k : k + 1],
                            zm_sb[:, 128 * j : 128 * (j + 1)],
                            u_sb[:],
                            start=True,
                            stop=True,
                        )
                    nc.vector.tensor_copy(rn_sb[:, 4 * i : 4 * (i + 1)], rnps[:])

            emit_t_piece(0)

            e_tiles = {}
            first_pe = pe_sum[0] if pe_sum else None
            first_dve = dve_sum[0] if dve_sum else None

            for j in range(NCHUNK + 1):
                if j < NCHUNK:
                    if j % 4 == 1 and j // 4 + 1 <= 7:
                        emit_t_piece(j // 4 + 1)
                    if j % 4 == 2 and j // 4 + 1 <= 7:
                        emit_vt_batch(j // 4 + 1)
                    # logits^T chunk j: (keys 128, queries 1024)
                    lps = lpool.tile([128, MBLK], F32, tag="L")
                    for h in range(2):
                        nc.tensor.matmul(
                            lps[:, h * 512 : (h + 1) * 512],
                            t_sb[:, 128 * j : 128 * (j + 1)],
                            zm_sb[:, h * 512 : (h + 1) * 512],
                            start=True,
                            stop=True,
                        )
                    ej = epool.tile([128, MBLK], F32R, tag="E")
                    bias = rn_sb[:, j : j + 1] if use_qk_bias else 0.0
                    nc.scalar.activation(ej[:], lps[:], AF.Exp, bias=bias)
                    e_tiles[j] = ej
                    if j == 0:
                        emit_vt_batch(0)
                if j >= 1:
                    jj = j - 1
                    ej = e_tiles.pop(jj)
                    for h in range(2):
                        nc.tensor.matmul(
                            out_ps[:, h * 512 : (h + 1) * 512],
                            vt_sb[:, 128 * jj : 128 * (jj + 1)],
                            ej[:, h * 512 : (h + 1) * 512],
                            start=(jj == 0),
                            stop=(jj == NCHUNK - 1),
                        )
                    if jj in pe_sum:
                        for h in range(2):
                            nc.tensor.matmul(
                                s_ps[0:1, h * 512 : (h + 1) * 512],
                                ones_col[:],
                                ej[:, h * 512 : (h + 1) * 512],
                                start=(jj == first_pe),
                                stop=False,
                                skip_group_check=True,
                            )
                    else:
                        if jj == first_dve:
                            nc.vector.tensor_copy(acc[:], ej[:])
                        else:
                            nc.vector.tensor_add(acc[:], acc[:], ej[:])

            # tail, in halves so ln/exp/broadcast/final/DMA pipeline
            for h in range(2):
                sl = slice(h * 512, (h + 1) * 512)
                ph = slice(0, 1)
                # fold the DVE accumulator into s (cross-partition reduce)
                nc.tensor.matmul(
                    s_ps[0:1, sl],
                    ones_col[:],
                    acc[:, sl],
                    start=(first_pe is None),
                    stop=True,
                    skip_group_check=True,
                )
                # r = 1/s via exp(-ln s): same ACT table set as the main exps
                nc.scalar.activation(lns[:, sl], s_ps[:, sl], AF.Ln)
                nc.scalar.activation(rvec[:, sl], lns[:, sl], AF.Exp, scale=-1.0)
                # broadcast r across partitions with a K=1 matmul, fold gamma
                rb_ps = lpool.tile([128, 512], F32, tag="L")
                nc.tensor.matmul(
                    rb_ps[:], ones_row[:], rvec[:, sl], start=True, stop=True
                )
                nc.vector.tensor_scalar(
                    out=rb_sb[:, sl],
                    in0=rb_ps[:],
                    scalar1=gam_sb[:, 0:1],
                    scalar2=None,
                    op0=ALU.mult,
                )
                # out = zc + (outPV * gamma/s + gamma*bv)
                nc.vector.tensor_tensor(
                    tmp_sb[:, sl], out_ps[:, sl], rb_sb[:, sl], op=ALU.mult
                )
                nc.vector.scalar_tensor_tensor(
                    out_sb[:, sl],
                    tmp_sb[:, sl],
                    adv_sb[:, 0:1],
                    zc_sb[:, sl],
                    op0=ALU.add,
                    op1=ALU.add,
                )
                nc.sync.dma_start(out_d[:, sl], out_sb[:, sl])

    nc.compile()
    return nc


_CACHE = {}


def _get_program(use_qk_bias: bool):
    if use_qk_bias not in _CACHE:
        _CACHE[use_qk_bias] = _build(use_qk_bias)
    return _CACHE[use_qk_bias]


def kernel(zc, zm, Wq, bq, Wk, bk, Wv, bv, gamma):
    global LAST_RESULTS
    zc = np.ascontiguousarray(zc, dtype=np.float32)
    zm = np.ascontiguousarray(zm, dtype=np.float32)
    zmf = zm.reshape(B, CM, N)
    zcf = zc.reshape(B, CC, N)

    Wq = np.asarray(Wq, dtype=np.float32)
    Wk = np.asarray(Wk, dtype=np.float32)
    Wv = np.asarray(Wv, dtype=np.float32)
    gt = (Wk.astype(np.float64).T @ Wq.astype(np.float64)).astype(np.float32)
    wvt = np.ascontiguousarray(Wv.T).astype(ml_dtypes.bfloat16)
    gamma_v = np.float32(np.asarray(gamma).reshape(-1)[0])
    gam_arr = np.full((CC, 1), gamma_v, dtype=np.float32)
    adv_arr = (gamma_v * np.asarray(bv, dtype=np.float32)).reshape(CC, 1)
    adv_arr = np.ascontiguousarray(adv_arr)

    use_qk_bias = bool(np.any(bq)) or bool(np.any(bk))
    nc = _get_program(use_qk_bias)

    in_maps = []
    for c in range(NCORES):
        b, jblk = divmod(c, 4)
        m = {
            "zm": np.ascontiguousarray(np.roll(zmf[b], -MBLK * jblk, axis=1)),
            "zc": np.ascontiguousarray(zcf[b][:, MBLK * jblk : MBLK * (jblk + 1)]),
            "gt": gt,
            "wvt": wvt,
            "gam": gam_arr,
            "adv": adv_arr,
            "onesc": np.ones((128, 1), dtype=np.float32),
            "onesr": np.ones((1, 128), dtype=np.float32),
        }
        if use_qk_bias:
            m["u"] = np.ascontiguousarray(
                (Wk.T @ np.asarray(bq, dtype=np.float32)).reshape(CM, 1)
            )
        in_maps.append(m)

    trace = bool(int(os.environ.get("BASS_KERNEL_TRACE", "0")))
    if trace and not _ensure_ntff_hook():
        trace = False
    res = run_bass_kernel_spmd(
        nc,
        in_maps,
        core_ids=list(range(NCORES)),
        trace=trace,
    )
    LAST_RESULTS = res

    out = np.empty((B, CC, N), dtype=np.float32)
    for c in range(NCORES):
        b, jblk = divmod(c, 4)
        out[b][:, MBLK * jblk : MBLK * (jblk + 1)] = res.results[c]["out"]
    return out.reshape(zc.shape)


# revision 22
# speedup vs baseline: 1.0945x; 1.0892x over previous
"""Trainium2 Bass kernel for nn_AttentionAggregator3d.

Math (per batch b):
    zmf = zm.reshape(CM, N)                     # N = D*W*H = 4096 tokens
    q = Wq @ zmf + bq ; k = Wk @ zmf + bk       # (16, N)
    v = Wv @ zmf + bv                           # (128, N)
    A = softmax_n(q^T k)                        # (N, N), softmax over keys n
    out = v @ A^T ; result = zc + gamma * out

Key transformations used by the kernel:
  * logits = zmf^T G zmf (+ key-side bias term) with G = Wq^T Wk precomputed
    on host, turning the K=16 contraction into a full K=128 PE contraction.
  * bq/bk only affect softmax through the per-key term r[n] = (Wk^T bq)·zm[:,n]
    (per-query terms cancel in softmax); handled as a per-partition exp bias.
  * Sharding: 8 cores = batch (2) x query-block (4, 1024 queries each). Each
    core sees its batch's zm rotated so its query block sits at columns 0:1024
    (softmax/PV sum over all keys, so key order is irrelevant).
  * Layout: exp'd scores E^T are kept (keys on partitions, queries free) so
    the PV matmul contracts over keys on the PE, all in float32r (full-rate
    fp32-class matmuls). Softmax denominators are accumulated partly by PE
    ones-matmuls into a single PSUM bank (two m-halves parked on partitions
    0 and 32), partly by DVE adds (tunable split), folded, inverted via
    exp(-ln s) (kept inside one ACT table set), broadcast with a K=1 matmul,
    and applied with gamma / zc in a halved, pipelined DVE tail.
"""

import os
import sys
import types

import ml_dtypes
import numpy as np

import concourse.bacc as bacc_mod
import concourse.tile as tile
from concourse import mybir
from concourse.bass_utils import run_bass_kernel_spmd

B, CC, CM, P = 2, 128, 128, 16
N = 16 * 16 * 16          # 4096 tokens
MBLK = N // 4             # 1024 queries per core
NCORES = 8
NCHUNK = N // 128         # 32 key chunks of 128

F32 = mybir.dt.float32
F32R = mybir.dt.float32r
BF16 = mybir.dt.bfloat16
AF = mybir.ActivationFunctionType
ALU = mybir.AluOpType

# chunks whose softmax-denominator contribution is summed on the PE
# (ones-matmul); the rest accumulate on the DVE. Tunable.
PE_SUM_PERIOD = int(os.environ.get("BASS_PE_SUM_PERIOD", "2"))

LAST_RESULTS = None  # BassKernelResults of the most recent run (for test.py)


def _ensure_ntff_hook() -> bool:
    """The grading image lacks antenv.axon_hooks; synthesize it from the
    boot module's ctypes NTFF driver so trace=True works under axon."""
    try:
        import antenv.axon_hooks  # noqa: F401

        return True
    except ImportError:
        pass
    try:
        import antenv
        from trn_agent_boot.trn_boot import _ntff_profile_via_ctypes

        hook = _ntff_profile_via_ctypes("/opt/axon/libaxon_pjrt.so")
        mod = types.ModuleType("antenv.axon_hooks")
        mod.get_axon_ntff_profile_hook = lambda: hook
        mod.set_axon_ntff_profile_hook = lambda h: None
        sys.modules["antenv.axon_hooks"] = mod
        antenv.axon_hooks = mod
        return hook is not None
    except Exception:
        return False


# Route Exp and Ln to the one table set that holds both, so the kernel pays a
# single ACT_TABLE_LOAD instead of three (exp -> ln -> exp again).
_orig_gat = bacc_mod.get_activation_tables
_COMBINED_SET = "natural_log_exp_and_others"


def _patched_gat(arch):
    tabs = _orig_gat(arch)
    if _COMBINED_SET in tabs:
        for name, fns in tabs.items():
            if name != _COMBINED_SET:
                fns.discard(AF.Exp)
                fns.discard(AF.Ln)
    return tabs


bacc_mod.get_activation_tables = _patched_gat


if os.environ.get("BASS_LDW_OPT", "0") == "1":
    import subprocess as _sp

    _orig_run = _sp.run

    def _patched_run(cmd, *a, **kw):
        if isinstance(cmd, list) and any(
            isinstance(c, str) and "enable-ldw-opt" in c for c in cmd
        ):
            cmd = [
                c.replace("--enable-ldw-opt=false", "--enable-ldw-opt=true")
                if isinstance(c, str)
                else c
                for c in cmd
            ]
        return _orig_run(cmd, *a, **kw)

    _sp.run = _patched_run


def _build(use_qk_bias: bool):
    nc = bacc_mod.Bacc(
        "TRN2",
        target_bir_lowering=False,
        debug=False,
        num_devices=NCORES,
    )

    zm_d = nc.dram_tensor("zm", (CM, N), F32R, kind="ExternalInput").ap()
    zc_d = nc.dram_tensor("zc", (CC, MBLK), F32, kind="ExternalInput").ap()
    gt_d = nc.dram_tensor("gt", (CM, CM), F32R, kind="ExternalInput").ap()
    wvt_d = nc.dram_tensor("wvt", (CM, CC), BF16, kind="ExternalInput").ap()
    gam_d = nc.dram_tensor("gam", (CC, 1), F32, kind="ExternalInput").ap()
    adv_d = nc.dram_tensor("adv", (CC, 1), F32, kind="ExternalInput").ap()
    onesc_d = nc.dram_tensor("onesc", (128, 1), F32R, kind="ExternalInput").ap()
    onesr_d = nc.dram_tensor("onesr", (1, 128), F32R, kind="ExternalInput").ap()
    if use_qk_bias:
        u_d = nc.dram_tensor("u", (CM, 1), F32R, kind="ExternalInput").ap()
    out_d = nc.dram_tensor("out", (CC, MBLK), F32, kind="ExternalOutput").ap()

    pe_sum = [j for j in range(NCHUNK) if j % PE_SUM_PERIOD == PE_SUM_PERIOD - 1]
    dve_sum = [j for j in range(NCHUNK) if j not in pe_sum]

    with tile.TileContext(nc) as tc:
        with (
            tc.tile_pool(name="consts", bufs=1) as consts,
            tc.tile_pool(name="epool", bufs=6) as epool,
            tc.tile_pool(name="lpool", bufs=2, space="PSUM") as lpool,
            tc.tile_pool(name="opool", bufs=1, space="PSUM") as opool,
            tc.tile_pool(name="spool", bufs=1, space="PSUM") as spool,
        ):
            zm_sb = consts.tile([CM, N], F32R, tag="zm")
            zm_bf = consts.tile([CM, N], BF16, tag="zmbf")
            t_sb = consts.tile([CM, N], F32R, tag="t")
            vt_sb = consts.tile([128, N], F32R, tag="vt")  # chunk j at cols 128j
            zc_sb = consts.tile([CC, MBLK], F32, tag="zc")
            gt_sb = consts.tile([CM, CM], F32R, tag="gt")
            wvt_sb = consts.tile([CM, CC], BF16, tag="wvt")
            gam_sb = consts.tile([CC, 1], F32, tag="gam")
            adv_sb = consts.tile([CC, 1], F32, tag="adv")
            ones_col = consts.tile([128, 1], F32R, tag="onesc")
            ones_row = consts.tile([1, 128], F32R, tag="onesr")
            acc = consts.tile([128, MBLK], F32R, tag="acc")
            lns = consts.tile([1, MBLK], F32, tag="lns")
            rvec = consts.tile([1, MBLK], F32R, tag="rvec")
            rb_sb = consts.tile([128, MBLK], F32, tag="rb")
            tmp_sb = consts.tile([CC, MBLK], F32, tag="tmp")
            out_sb = consts.tile([CC, MBLK], F32, tag="outsb")
            if use_qk_bias:
                u_sb = consts.tile([CM, 1], F32R, tag="u")
                rn_sb = consts.tile([128, NCHUNK], F32, tag="rn")

            # ---- input DMAs, fanned across idle engine sequencers: each
            # dma_start costs ~0.6us of issue time on its sequencer, so
            # serializing them on one engine would delay the first matmul ----
            nc.scalar.dma_start(gt_sb[:], gt_d)
            nc.sync.dma_start(zm_sb[:, 0:512], zm_d[:, 0:512])
            nc.sync.dma_start(zm_sb[:, 512:1024], zm_d[:, 512:1024])
            nc.scalar.dma_start(ones_col[:], onesc_d)
            nc.gpsimd.dma_start(zm_sb[:, 1024:2048], zm_d[:, 1024:2048])
            nc.gpsimd.dma_start(zm_sb[:, 2048:3072], zm_d[:, 2048:3072])
            nc.gpsimd.dma_start(zm_sb[:, 3072:4096], zm_d[:, 3072:4096])
            nc.scalar.dma_start(wvt_sb[:], wvt_d)
            nc.sync.dma_start(ones_row[:], onesr_d)
            nc.sync.dma_start(gam_sb[:], gam_d)
            nc.sync.dma_start(adv_sb[:], adv_d)
            if use_qk_bias:
                nc.gpsimd.dma_start(u_sb[:], u_d)
            nc.sync.dma_start(zc_sb[:], zc_d)

            out_ps = opool.tile([CC, MBLK], F32, tag="out")
            # one PSUM bank: m-half h sums parked on partition 32h
            s_ps = spool.tile([1, MBLK], F32, tag="s")

            def emit_t_piece(i):
                # t[:, 512i:512(i+1)] = G @ zm[:, ...] (covers chunks 4i..4i+3)
                tps = lpool.tile([128, 512], F32, tag="L")
                nc.tensor.matmul(
                    tps[:],
                    gt_sb[:],
                    zm_sb[:, i * 512 : (i + 1) * 512],
                    start=True,
                    stop=True,
                )
                nc.scalar.copy(t_sb[:, i * 512 : (i + 1) * 512], tps[:])

            def emit_vt_batch(i):
                # vt chunk j = (zm chunk j)^T @ Wv^T for j in 4i..4i+3
                nc.vector.tensor_copy(
                    zm_bf[:, i * 512 : (i + 1) * 512],
                    zm_sb[:, i * 512 : (i + 1) * 512].bitcast(F32),
                )
                vps = lpool.tile([128, 512], F32, tag="L")
                for k in range(4):
                    j = 4 * i + k
                    nc.tensor.matmul(
                        vps[:, 128 * k : 128 * (k + 1)],
                        zm_bf[:, 128 * j : 128 * (j + 1)],
                        wvt_sb[:],
                        start=True,
                        stop=True,
                    )
                nc.vector.tensor_copy(vt_sb[:, i * 512 : (i + 1) * 512], vps[:])
                if use_qk_bias:
                    rnps = lpool.tile([128, 4], F32, tag="L")
                    for k in range(4):
                        j = 4 * i + k
                        nc.tensor.matmul(
                            rnps[:, k : k + 1],
                            zm_sb[:, 128 * j : 128 * (j + 1)],
                            u_sb[:],
                            start=True,
                            stop=True,
                        )
                    nc.vector.tensor_copy(rn_sb[:, 4 * i : 4 * (i + 1)], rnps[:])

            emit_t_piece(0)

            e_tiles = {}
            first_pe = pe_sum[0] if pe_sum else None
            first_dve = dve_sum[0] if dve_sum else None

            LAG = 2
            for j in range(NCHUNK + LAG):
                if j < NCHUNK:
                    if j % 4 == 1 and j // 4 + 1 <= 7:
                        emit_t_piece(j // 4 + 1)
                    if j % 4 == 2 and j // 4 + 1 <= 7:
                        emit_vt_batch(j // 4 + 1)
                    # logits^T chunk j: (keys 128, queries 1024)
                    lps = lpool.tile([128, MBLK], F32, tag="L")
                    for h in range(2):
                        nc.tensor.matmul(
                            lps[:, h * 512 : (h + 1) * 512],
                            t_sb[:, 128 * j : 128 * (j + 1)],
                            zm_sb[:, h * 512 : (h + 1) * 512],
                            start=True,
                            stop=True,
                        )
                    ej = epool.tile([128, MBLK], F32R, tag="E")
                    bias = rn_sb[:, j : j + 1] if use_qk_bias else 0.0
                    nc.scalar.activation(ej[:], lps[:], AF.Exp, bias=bias)
                    e_tiles[j] = ej
                    if j == 0:
                        emit_vt_batch(0)
                if j >= LAG:
                    jj = j - LAG
                    ej = e_tiles.pop(jj)
                    for h in range(2):
                        nc.tensor.matmul(
                            out_ps[:, h * 512 : (h + 1) * 512],
                            vt_sb[:, 128 * jj : 128 * (jj + 1)],
                            ej[:, h * 512 : (h + 1) * 512],
                            start=(jj == 0),
                            stop=(jj == NCHUNK - 1),
                        )
                    if jj in pe_sum:
                        for h in range(2):
                            nc.tensor.matmul(
                                s_ps[0:1, h * 512 : (h + 1) * 512],
                                ones_col[:],
                                ej[:, h * 512 : (h + 1) * 512],
                                start=(jj == first_pe),
                                stop=False,
                                skip_group_check=True,
                            )
                    else:
                        if jj == first_dve:
                            nc.vector.tensor_copy(acc[:], ej[:])
                        else:
                            nc.vector.tensor_add(acc[:], acc[:], ej[:])

            # tail, in halves so ln/exp/broadcast/final/DMA pipeline
            for h in range(2):
                sl = slice(h * 512, (h + 1) * 512)
                ph = slice(0, 1)
                # fold the DVE accumulator into s (cross-partition reduce)
                nc.tensor.matmul(
                    s_ps[0:1, sl],
                    ones_col[:],
                    acc[:, sl],
                    start=(first_pe is None),
                    stop=True,
                    skip_group_check=True,
                )
                # r = 1/s via exp(-ln s): same ACT table set as the main exps
                nc.scalar.activation(lns[:, sl], s_ps[:, sl], AF.Ln)
                nc.scalar.activation(rvec[:, sl], lns[:, sl], AF.Exp, scale=-1.0)
                # broadcast r across partitions with a K=1 matmul, fold gamma
                rb_ps = lpool.tile([128, 512], F32, tag="L")
                nc.tensor.matmul(
                    rb_ps[:], ones_row[:], rvec[:, sl], start=True, stop=True
                )
                nc.vector.tensor_scalar(
                    out=rb_sb[:, sl],
                    in0=rb_ps[:],
                    scalar1=gam_sb[:, 0:1],
                    scalar2=None,
                    op0=ALU.mult,
                )
                # out = zc + (outPV * gamma/s + gamma*bv)
                nc.vector.tensor_tensor(
                    tmp_sb[:, sl], out_ps[:, sl], rb_sb[:, sl], op=ALU.mult
                )
                nc.vector.scalar_tensor_tensor(
                    out_sb[:, sl],
                    tmp_sb[:, sl],
                    adv_sb[:, 0:1],
                    zc_sb[:, sl],
                    op0=ALU.add,
                    op1=ALU.add,
                )
                nc.sync.dma_start(out_d[:, sl], out_sb[:, sl])

    nc.compile()
    return nc


_CACHE = {}


def _get_program(use_qk_bias: bool):
    if use_qk_bias not in _CACHE:
        _CACHE[use_qk_bias] = _build(use_qk_bias)
    return _CACHE[use_qk_bias]


def kernel(zc, zm, Wq, bq, Wk, bk, Wv, bv, gamma):
    global LAST_RESULTS
    zc = np.ascontiguousarray(zc, dtype=np.float32)
    zm = np.ascontiguousarray(zm, dtype=np.float32)
    zmf = zm.reshape(B, CM, N)
    zcf = zc.reshape(B, CC, N)

    Wq = np.asarray(Wq, dtype=np.float32)
    Wk = np.asarray(Wk, dtype=np.float32)
    Wv = np.asarray(Wv, dtype=np.float32)
    gt = (Wk.astype(np.float64).T @ Wq.astype(np.float64)).astype(np.float32)
    wvt = np.ascontiguousarray(Wv.T).astype(ml_dtypes.bfloat16)
    gamma_v = np.float32(np.asarray(gamma).reshape(-1)[0])
    gam_arr = np.full((CC, 1), gamma_v, dtype=np.float32)
    adv_arr = (gamma_v * np.asarray(bv, dtype=np.float32)).reshape(CC, 1)
    adv_arr = np.ascontiguousarray(adv_arr)

    use_qk_bias = bool(np.any(bq)) or bool(np.any(bk))
    nc = _get_program(use_qk_bias)

    in_maps = []
    for c in range(NCORES):
        b, jblk = divmod(c, 4)
        m = {
            "zm": np.ascontiguousarray(np.roll(zmf[b], -MBLK * jblk, axis=1)),
            "zc": np.ascontiguousarray(zcf[b][:, MBLK * jblk : MBLK * (jblk + 1)]),
            "gt": gt,
            "wvt": wvt,
            "gam": gam_arr,
            "adv": adv_arr,
            "onesc": np.ones((128, 1), dtype=np.float32),
            "onesr": np.ones((1, 128), dtype=np.float32),
        }
        if use_qk_bias:
            m["u"] = np.ascontiguousarray(
                (Wk.T @ np.asarray(bq, dtype=np.float32)).reshape(CM, 1)
            )
        in_maps.append(m)

    trace = bool(int(os.environ.get("BASS_KERNEL_TRACE", "0")))
    if trace and not _ensure_ntff_hook():
        trace = False
    res = run_bass_kernel_spmd(
        nc,
        in_maps,
        core_ids=list(range(NCORES)),
        trace=trace,
    )
    LAST_RESULTS = res

    out = np.empty((B, CC, N), dtype=np.float32)
    for c in range(NCORES):
        b, jblk = divmod(c, 4)
        out[b][:, MBLK * jblk : MBLK * (jblk + 1)] = res.results[c]["out"]
    return out.reshape(zc.shape)


# revision 40
# speedup vs baseline: 1.1734x; 1.0721x over previous
"""Trainium2 Bass kernel for nn_AttentionAggregator3d.

Math (per batch b):
    zmf = zm.reshape(CM, N)                     # N = D*W*H = 4096 tokens
    q = Wq @ zmf + bq ; k = Wk @ zmf + bk       # (16, N)
    v = Wv @ zmf + bv                           # (128, N)
    A = softmax_n(q^T k)                        # (N, N), softmax over keys n
    out = v @ A^T ; result = zc + gamma * out

Key transformations used by the kernel:
  * logits = zmf^T G zmf (+ key-side bias term) with G = Wq^T Wk precomputed
    on host, turning the K=16 contraction into a full K=128 PE contraction.
  * bq/bk only affect softmax through the per-key term r[n] = (Wk^T bq)·zm[:,n]
    (per-query terms cancel in softmax); handled as a per-partition exp bias.
  * Sharding: 8 cores = batch (2) x query-block (4, 1024 queries each). Each
    core sees its batch's zm rotated so its query block sits at columns 0:1024
    (softmax/PV sum over all keys, so key order is irrelevant).
  * Layout: exp'd scores E^T are kept (keys on partitions, queries free) so
    the PV matmul contracts over keys on the PE in float32r (full-rate
    fp32-class matmuls); the value projection runs in bf16 (fast weight
    load). Softmax denominators are split by query-half: PE ones-matmuls
    accumulate half 0 into a single PSUM bank while DVE adds accumulate
    half 1 in SBUF (folded by one matmul at the end), keeping a PSUM bank
    free for t/v staging. 1/s is computed as exp(-ln s) inside one ACT
    table set, broadcast with a K=1 matmul, and applied with gamma / zc in
    a quartered, pipelined DVE tail. The PV matmuls trail the logits
    pipeline by two chunks so the PE never waits on the exponentials.
"""

import os
import sys
import types

import ml_dtypes
import numpy as np

import concourse.bacc as bacc_mod
import concourse.tile as tile
from concourse import mybir
from concourse.bass_utils import run_bass_kernel_spmd

B, CC, CM, P = 2, 128, 128, 16
N = 16 * 16 * 16          # 4096 tokens
MBLK = N // 4             # 1024 queries per core
NCORES = 8
NCHUNK = N // 128         # 32 key chunks of 128

F32 = mybir.dt.float32
F32R = mybir.dt.float32r
BF16 = mybir.dt.bfloat16
AF = mybir.ActivationFunctionType
ALU = mybir.AluOpType

LAST_RESULTS = None  # BassKernelResults of the most recent run (for test.py)


def _ensure_ntff_hook() -> bool:
    """The grading image lacks antenv.axon_hooks; synthesize it from the
    boot module's ctypes NTFF driver so trace=True works under axon."""
    try:
        import antenv.axon_hooks  # noqa: F401

        return True
    except ImportError:
        pass
    try:
        import antenv
        from trn_agent_boot.trn_boot import _ntff_profile_via_ctypes

        hook = _ntff_profile_via_ctypes("/opt/axon/libaxon_pjrt.so")
        mod = types.ModuleType("antenv.axon_hooks")
        mod.get_axon_ntff_profile_hook = lambda: hook
        mod.set_axon_ntff_profile_hook = lambda h: None
        sys.modules["antenv.axon_hooks"] = mod
        antenv.axon_hooks = mod
        return hook is not None
    except Exception:
        return False


# Route Exp and Ln to the one table set that holds both, so the kernel pays a
# single ACT_TABLE_LOAD instead of three (exp -> ln -> exp again).
_orig_gat = bacc_mod.get_activation_tables
_COMBINED_SET = "natural_log_exp_and_others"


def _patched_gat(arch):
    tabs = _orig_gat(arch)
    if _COMBINED_SET in tabs:
        for name, fns in tabs.items():
            if name != _COMBINED_SET:
                fns.discard(AF.Exp)
                fns.discard(AF.Ln)
    return tabs


bacc_mod.get_activation_tables = _patched_gat


def _build(use_qk_bias: bool):
    nc = bacc_mod.Bacc(
        "TRN2",
        target_bir_lowering=False,
        debug=False,
        num_devices=NCORES,
    )

    zm_d = nc.dram_tensor("zm", (CM, N), F32R, kind="ExternalInput").ap()
    zc_d = nc.dram_tensor("zc", (CC, MBLK), F32, kind="ExternalInput").ap()
    gt_d = nc.dram_tensor("gt", (CM, CM), F32R, kind="ExternalInput").ap()
    wvt_d = nc.dram_tensor("wvt", (CM, CC), BF16, kind="ExternalInput").ap()
    gam_d = nc.dram_tensor("gam", (CC, 1), F32, kind="ExternalInput").ap()
    adv_d = nc.dram_tensor("adv", (CC, 1), F32, kind="ExternalInput").ap()
    onesc_d = nc.dram_tensor("onesc", (128, 1), F32R, kind="ExternalInput").ap()
    onesr_d = nc.dram_tensor("onesr", (1, 128), F32R, kind="ExternalInput").ap()
    if use_qk_bias:
        u_d = nc.dram_tensor("u", (CM, 1), F32R, kind="ExternalInput").ap()
    out_d = nc.dram_tensor("out", (CC, MBLK), F32, kind="ExternalOutput").ap()

    with tile.TileContext(nc) as tc:
        with (
            tc.tile_pool(name="consts", bufs=1) as consts,
            tc.tile_pool(name="epool", bufs=8) as epool,
            tc.tile_pool(name="lpool", bufs=2, space="PSUM") as lpool,
            tc.tile_pool(name="tpool", bufs=1, space="PSUM") as tpool,
            tc.tile_pool(name="opool", bufs=1, space="PSUM") as opool,
            tc.tile_pool(name="spool", bufs=1, space="PSUM") as spool,
        ):
            zm_sb = consts.tile([CM, N], F32R, tag="zm")
            zm_bf = consts.tile([CM, N], BF16, tag="zmbf")
            t_sb = consts.tile([CM, N], F32R, tag="t")
            vt_sb = consts.tile([128, N], F32R, tag="vt")  # chunk j at cols 128j
            zc_sb = consts.tile([CC, MBLK], F32, tag="zc")
            gt_sb = consts.tile([CM, CM], F32R, tag="gt")
            wvt_sb = consts.tile([CM, CC], BF16, tag="wvt")
            gam_sb = consts.tile([CC, 1], F32, tag="gam")
            adv_sb = consts.tile([CC, 1], F32, tag="adv")
            ones_col = consts.tile([128, 1], F32R, tag="onesc")
            ones_row = consts.tile([1, 128], F32R, tag="onesr")
            acc = consts.tile([128, 512], F32R, tag="acc")
            acc0 = consts.tile([128, 512], F32R, tag="acc0")
            accg = consts.tile([128, 512], F32R, tag="accg")
            lns = consts.tile([1, MBLK], F32, tag="lns")
            rvec = consts.tile([1, MBLK], F32R, tag="rvec")
            rb_sb = consts.tile([128, MBLK], F32, tag="rb")
            tmp_sb = consts.tile([CC, MBLK], F32, tag="tmp")
            out_sb = consts.tile([CC, MBLK], F32, tag="outsb")
            if use_qk_bias:
                u_sb = consts.tile([CM, 1], F32R, tag="u")
                rn_sb = consts.tile([128, NCHUNK], F32, tag="rn")

            # ---- input DMAs, fanned across idle engine sequencers: each
            # dma_start costs ~0.6us of issue time on its sequencer, so
            # serializing them on one engine would delay the first matmul ----
            nc.scalar.dma_start(gt_sb[:], gt_d)
            nc.sync.dma_start(zm_sb[:, 0:256], zm_d[:, 0:256])
            nc.scalar.dma_start(zm_sb[:, 256:512], zm_d[:, 256:512])
            nc.sync.dma_start(zm_sb[:, 512:768], zm_d[:, 512:768])
            nc.scalar.dma_start(zm_sb[:, 768:1024], zm_d[:, 768:1024])
            nc.scalar.dma_start(ones_col[:], onesc_d)
            nc.gpsimd.dma_start(zm_sb[:, 1024:2048], zm_d[:, 1024:2048])
            nc.gpsimd.dma_start(zm_sb[:, 2048:3072], zm_d[:, 2048:3072])
            nc.gpsimd.dma_start(zm_sb[:, 3072:4096], zm_d[:, 3072:4096])
            nc.scalar.dma_start(wvt_sb[:], wvt_d)
            nc.sync.dma_start(ones_row[:], onesr_d)
            nc.sync.dma_start(gam_sb[:], gam_d)
            nc.sync.dma_start(adv_sb[:], adv_d)
            if use_qk_bias:
                nc.gpsimd.dma_start(u_sb[:], u_d)
            nc.sync.dma_start(zc_sb[:], zc_d)

            out_ps = opool.tile([CC, MBLK], F32, tag="out")
            # one PSUM bank: m-half h sums parked on partition 32h
            s_ps = spool.tile([1, 512], F32, tag="s")

            def emit_t_piece(i):
                # t[:, 512i:512(i+1)] = G @ zm[:, ...] (covers chunks 4i..4i+3)
                tps = tpool.tile([128, 512], F32, tag="T")
                nc.tensor.matmul(
                    tps[:],
                    gt_sb[:],
                    zm_sb[:, i * 512 : (i + 1) * 512],
                    start=True,
                    stop=True,
                )
                nc.scalar.copy(t_sb[:, i * 512 : (i + 1) * 512], tps[:])

            def emit_vt_batch(i):
                # vt chunk j = (zm chunk j)^T @ Wv^T for j in 4i..4i+3
                nc.vector.tensor_copy(
                    zm_bf[:, i * 512 : (i + 1) * 512],
                    zm_sb[:, i * 512 : (i + 1) * 512].bitcast(F32),
                )
                vps = tpool.tile([128, 512], F32, tag="T")
                for k in range(4):
                    j = 4 * i + k
                    nc.tensor.matmul(
                        vps[:, 128 * k : 128 * (k + 1)],
                        zm_bf[:, 128 * j : 128 * (j + 1)],
                        wvt_sb[:],
                        start=True,
                        stop=True,
                    )
                nc.vector.tensor_copy(vt_sb[:, i * 512 : (i + 1) * 512], vps[:])
                if use_qk_bias:
                    rnps = tpool.tile([128, 4], F32, tag="T")
                    for k in range(4):
                        j = 4 * i + k
                        nc.tensor.matmul(
                            rnps[:, k : k + 1],
                            zm_sb[:, 128 * j : 128 * (j + 1)],
                            u_sb[:],
                            start=True,
                            stop=True,
                        )
                    nc.vector.tensor_copy(rn_sb[:, 4 * i : 4 * (i + 1)], rnps[:])

            emit_t_piece(0)

            e_tiles = {}

            LAG = int(os.environ.get("BASS_PV_LAG", "2"))
            for j in range(NCHUNK + LAG):
                if j < NCHUNK:
                    if j % 4 == 1 and j // 4 + 1 <= 7:
                        emit_t_piece(j // 4 + 1)
                    if j % 4 == 2 and j // 4 + 1 <= 7:
                        emit_vt_batch(j // 4 + 1)
                    # logits^T chunk j: (keys 128, queries 1024)
                    lps = lpool.tile([128, MBLK], F32, tag="L")
                    for h in range(2):
                        nc.tensor.matmul(
                            lps[:, h * 512 : (h + 1) * 512],
                            t_sb[:, 128 * j : 128 * (j + 1)],
                            zm_sb[:, h * 512 : (h + 1) * 512],
                            start=True,
                            stop=True,
                        )
                    ej = epool.tile([128, MBLK], F32R, tag="E")
                    bias = rn_sb[:, j : j + 1] if use_qk_bias else 0.0
                    nc.scalar.activation(ej[:], lps[:], AF.Exp, bias=bias)
                    e_tiles[j] = ej
                    if j == 0:
                        emit_vt_batch(0)
                if j >= LAG:
                    jj = j - LAG
                    ej = e_tiles.pop(jj)
                    for h in range(2):
                        nc.tensor.matmul(
                            out_ps[:, h * 512 : (h + 1) * 512],
                            vt_sb[:, 128 * jj : 128 * (jj + 1)],
                            ej[:, h * 512 : (h + 1) * 512],
                            start=(jj == 0),
                            stop=(jj == NCHUNK - 1),
                        )
                    # three-way softmax-denominator split: PE ones-matmuls
                    # for most half-0 chunks, DVE adds for half 1 (+ some
                    # half-0), GPSIMD adds for a third of half 1.
                    if jj % 3 == 2:
                        if jj == 2:
                            nc.vector.tensor_copy(acc0[:], ej[:, 0:512])
                        else:
                            nc.vector.tensor_add(acc0[:], acc0[:], ej[:, 0:512])
                    else:
                        nc.tensor.matmul(
                            s_ps[0:1, :],
                            ones_col[:],
                            ej[:, 0:512],
                            start=(jj == 0),
                            stop=False,
                            skip_group_check=True,
                        )
                    if jj % 3 == 1:
                        if jj == 1:
                            nc.gpsimd.tensor_copy(accg[:], ej[:, 512:1024])
                        else:
                            nc.gpsimd.tensor_add(accg[:], accg[:], ej[:, 512:1024])
                    else:
                        if jj == 0:
                            nc.vector.tensor_copy(acc[:], ej[:, 512:1024])
                        else:
                            nc.vector.tensor_add(acc[:], acc[:], ej[:, 512:1024])

            # tail, in halves so ln/exp/broadcast/final/DMA pipeline
            # tail in 256-wide quarters so the ln/exp/broadcast/final/DMA
            # chains of successive quarters overlap across engines
            for q in range(4):
                sl = slice(q * 256, (q + 1) * 256)
                if q < 2:
                    if q == 0:
                        nc.tensor.matmul(
                            s_ps[0:1, :],
                            ones_col[:],
                            acc0[:],
                            start=False,
                            stop=True,
                            skip_group_check=True,
                        )
                    s_src = s_ps[0:1, q * 256 : (q + 1) * 256]
                else:
                    # fold the DVE + GPSIMD accumulators (cross-partition)
                    sfold = tpool.tile([1, 256], F32, tag="T")
                    qs = slice((q - 2) * 256, (q - 1) * 256)
                    nc.tensor.matmul(
                        sfold[:], ones_col[:], acc[:, qs], start=True, stop=False
                    )
                    nc.tensor.matmul(
                        sfold[:], ones_col[:], accg[:, qs], start=False, stop=True
                    )
                    s_src = sfold[:]
                # r = 1/s via exp(-ln s): same ACT table set as the main exps
                nc.scalar.activation(lns[:, sl], s_src, AF.Ln)
                nc.scalar.activation(rvec[:, sl], lns[:, sl], AF.Exp, scale=-1.0)
                # broadcast r across partitions with a K=1 matmul, fold gamma
                rb_ps = tpool.tile([128, 256], F32, tag="T")
                nc.tensor.matmul(
                    rb_ps[:], ones_row[:], rvec[:, sl], start=True, stop=True
                )
                nc.vector.tensor_scalar(
                    out=rb_sb[:, sl],
                    in0=rb_ps[:],
                    scalar1=gam_sb[:, 0:1],
                    scalar2=None,
                    op0=ALU.mult,
                )
                # out = zc + (outPV * gamma/s + gamma*bv)
                nc.vector.tensor_tensor(
                    tmp_sb[:, sl], out_ps[:, sl], rb_sb[:, sl], op=ALU.mult
                )
                nc.vector.scalar_tensor_tensor(
                    out_sb[:, sl],
                    tmp_sb[:, sl],
                    adv_sb[:, 0:1],
                    zc_sb[:, sl],
                    op0=ALU.add,
                    op1=ALU.add,
                )
                nc.sync.dma_start(out_d[:, sl], out_sb[:, sl])

    nc.compile()
    return nc


_CACHE = {}


def _get_program(use_qk_bias: bool):
    if use_qk_bias not in _CACHE:
        _CACHE[use_qk_bias] = _build(use_qk_bias)
    return _CACHE[use_qk_bias]


def kernel(zc, zm, Wq, bq, Wk, bk, Wv, bv, gamma):
    global LAST_RESULTS
    zc = np.ascontiguousarray(zc, dtype=np.float32)
    zm = np.ascontiguousarray(zm, dtype=np.float32)
    zmf = zm.reshape(B, CM, N)
    zcf = zc.reshape(B, CC, N)

    Wq = np.asarray(Wq, dtype=np.float32)
    Wk = np.asarray(Wk, dtype=np.float32)
    Wv = np.asarray(Wv, dtype=np.float32)
    gt = (Wk.astype(np.float64).T @ Wq.astype(np.float64)).astype(np.float32)
    wvt = np.ascontiguousarray(Wv.T).astype(ml_dtypes.bfloat16)
    gamma_v = np.float32(np.asarray(gamma).reshape(-1)[0])
    gam_arr = np.full((CC, 1), gamma_v, dtype=np.float32)
    adv_arr = (gamma_v * np.asarray(bv, dtype=np.float32)).reshape(CC, 1)
    adv_arr = np.ascontiguousarray(adv_arr)

    use_qk_bias = bool(np.any(bq)) or bool(np.any(bk))
    nc = _get_program(use_qk_bias)

    in_maps = []
    for c in range(NCORES):
        b, jblk = divmod(c, 4)
        m = {
            "zm": np.ascontiguousarray(np.roll(zmf[b], -MBLK * jblk, axis=1)),
            "zc": np.ascontiguousarray(zcf[b][:, MBLK * jblk : MBLK * (jblk + 1)]),
            "gt": gt,
            "wvt": wvt,
            "gam": gam_arr,
            "adv": adv_arr,
            "onesc": np.ones((128, 1), dtype=np.float32),
            "onesr": np.ones((1, 128), dtype=np.float32),
        }
        if use_qk_bias:
            m["u"] = np.ascontiguousarray(
                (Wk.T @ np.asarray(bq, dtype=np.float32)).reshape(CM, 1)
            )
        in_maps.append(m)

    trace = bool(int(os.environ.get("BASS_KERNEL_TRACE", "0")))
    if trace and not _ensure_ntff_hook():
        trace = False
    res = run_bass_kernel_spmd(
        nc,
        in_maps,
        core_ids=list(range(NCORES)),
        trace=trace,
    )
    LAST_RESULTS = res

    out = np.empty((B, CC, N), dtype=np.float32)
    for c in range(NCORES):
        b, jblk = divmod(c, 4)
        out[b][:, MBLK * jblk : MBLK * (jblk + 1)] = res.results[c]["out"]
    return out.reshape(zc.shape)
